# revision 12
# baseline (speedup 1.0000x reference)
"""Trainium2 Bass kernel for nn_ModelIAS_53618371724066 (segment_reduce).

Computes, for each batch row b:
    logits = hidden[b, 1:, :] @ W + b_vec          # [T, S]
    merged[w, :] = mean over {t : seg[b,t] == w} of logits[t, :]   (0 if empty)
    out[b] = merged.T                               # [S, T]

Strategy (data-parallel over batch, 32 rows per core on 8 cores):
  - hidden is host-transposed to [p, row, k, t] and quantized to fp8 e3m4
    (4 mantissa bits; |h| < 15.5 so range is safe): HALVES input HBM
    traffic and measures rel_err 1.41e-2, inside the 2e-2 gate.  W stays
    fp16 (fp8 W pushes the error past the gate).
  - Warm steady state is PE-stream-bound: 16 matmuls/row x ~58ns
    (N=130 columns at 2.4GHz, LDWEIGHTS hidden by FWL+shadow plane)
    = 928ns/row.  Everything else is scheduled to stay under that:
      DVE: 2x Mg tensor_scalar (fp16 iota input) + lsb PSUM->SBUF copy
      ACT: output PSUM->SBUF fp16 cast only
      GpSimd: hidden DMA issue, batched 2 rows per descriptor set
      Sync: W/seg/g/iota constants + output DMAs
  - The HAM clock gate needs ~3us of CONTINUOUS PE busy to unthrottle
    0.65->1.2->2.4GHz; any idle gap resets it.  So: (a) a few junk
    matmuls on a memset tile bridge the initial DMA wait, (b) hidden is
    prefetched 3 batches (6 rows) deep so the PE never starves, (c) the
    first fetch is split so row0's k0 chunk + W's k0 chunk land first
    (on the otherwise-idle sync/HWDGE ring) and the PE starts ~1us
    earlier.
  - Stage 1 (PE): logits[t_chunk, s] accumulated over 6 k-chunks into one
    PSUM tile [128, 2, S] fp32; bias folded as rank-1 matmul if nonzero.
  - Stage 2 (PE): out[w, s] = sum_c Mg[:, c, wchunk].T @ lsb[:, c, :],
    Mg stationary (built by DVE as (seg==w)*g in fp16), fp16 logits
    moving; emitted AFTER stage 1 of the next row (1-row software
    pipeline) so the in-order PE never waits on the lsb copy.
  - Output: ACT casts PSUM->SBUF fp16 per row into a 2-row tile; DMA out
    every 2 rows on the sync ring (the last pair goes per-row to shrink
    the end-of-kernel drain).  Host reassembles [w, s] -> [s, t].
  - Per-instruction sem-waits are legalized for the pinned walrus by
    _split_sync_waits.
"""

import numpy as np

import concourse.bass as bass
import concourse.tile as tile
from concourse import mybir
from concourse.bass_utils import run_bass_kernel_spmd

B, T, H, S = 256, 256, 768, 130
N_CORES = 8
RPC = B // N_CORES  # rows per core
KCH = H // 128  # k chunks of the hidden dim
F32 = mybir.dt.float32
HP = mybir.dt.float16
H8 = mybir.dt.float8e3  # e3m4: 4 mantissa bits, covers |h|<~15.5


def _split_sync_waits(nc):
    """The pinned walrus build rejects instructions carrying more than one
    sync-wait command ("Too many sync wait commands", setupSyncWait).  Keep
    one wait per instruction and hoist the rest onto NoOps inserted just
    before it on the same engine (same semantics: all waits still execute
    before the instruction, in stream order)."""
    for f in nc.m.functions:
        for blk in f.blocks:
            il = blk.instructions
            i = 0
            while i < len(il):
                inst = il[i]
                si = inst.sync_info
                if si is not None and si.on_wait and len(si.on_wait) >= 2:
                    waits = list(si.on_wait)
                    keep = [waits.pop()]
                    pos = i
                    for j, w in enumerate(waits):
                        nop = mybir.InstNoOp(name=f"{inst.name}_ws{j}", ins=[], outs=[])
                        nop.engine = inst.engine
                        nop.sync_info = mybir.SyncInfo(on_wait=[w], on_update=[])
                        il.insert(pos, nop)
                        pos += 1
                        i += 1
                    inst.sync_info = mybir.SyncInfo(
                        on_wait=keep, on_update=list(si.on_update)
                    )
                i += 1


def _build_program(rpc=RPC, with_bias=False, hid_bufs=10, n_junk=6, split_waits=True):
    nc = bass.Bass("TRN2", target_bir_lowering=False, debug=False)

    hid = nc.dram_tensor("hiddent", [128, rpc, KCH, T], H8, kind="ExternalInput")
    w_d = nc.dram_tensor("w", [128, KCH, S], HP, kind="ExternalInput")
    b_d = nc.dram_tensor("bvec", [1, S], HP, kind="ExternalInput")
    seg_d = nc.dram_tensor("segt", [128, 2, rpc], F32, kind="ExternalInput")
    g_d = nc.dram_tensor("gt", [128, 2, rpc], F32, kind="ExternalInput")
    iota_d = nc.dram_tensor("iota16", [128, T], HP, kind="ExternalInput")
    # [w_partition, row, w_chunk, s] fp16; host reassembles to [B, S, T]
    out_d = nc.dram_tensor("out", [128, rpc, 2, S], HP, kind="ExternalOutput")

    eq = mybir.AluOpType.is_equal
    mult = mybir.AluOpType.mult
    assert rpc % 2 == 0
    nbatch = rpc // 2
    with tile.TileContext(nc) as tc:
        with (
            tc.tile_pool(name="const", bufs=1) as const_pool,
            tc.tile_pool(name="hid", bufs=hid_bufs) as hid_pool,
            tc.tile_pool(name="mbar", bufs=3) as m_pool,
            tc.tile_pool(name="lsb", bufs=3) as l_pool,
            tc.tile_pool(name="osb", bufs=3) as o_pool,
            tc.tile_pool(name="psl", bufs=3, space=bass.MemorySpace.PSUM) as psl_pool,
            tc.tile_pool(name="pso", bufs=4, space=bass.MemorySpace.PSUM) as pso_pool,
            tc.tile_pool(name="psj", bufs=1, space=bass.MemorySpace.PSUM) as psj_pool,
        ):
            # --- junk warm-up matmuls: keep the PE continuously busy from
            # t~0 so the HAM clock ramp (needs 3us of uninterrupted busy)
            # starts before row 0's data lands.  Inputs are memset zeros;
            # output goes to a scratch PSUM bank.
            bats = {}
            obs = {}
            wt = const_pool.tile([128, KCH, S], HP)
            if n_junk:
                jw = const_pool.tile([128, 128], HP)
                nc.vector.memset(jw[:], 0.0)
                jm = const_pool.tile([128, 256], HP)
                nc.vector.memset(jm[:], 0.0)
                psj = psj_pool.tile([128, 256], F32)
                for _ in range(n_junk):
                    nc.tensor.matmul(psj[:], jw[:], jm[:], start=True, stop=True)

            # --- head start.  The HWDGE rings (sync/scalar) move bytes
            # ~10x slower than the gpsimd SWDGE ring for these small-run
            # shapes, so hidden AND W ride SWDGE; only the tiny constants
            # (iota/seg/g, needed ~3us in) go on the sync ring whose issue
            # runs in parallel with gpsimd's. ---
            def fetch_row(rr_, chunks=((0, KCH),)):
                t = hid_pool.tile([128, KCH, T], H8, tag="ht", name="ht")
                for j0, j1 in chunks:
                    nc.gpsimd.dma_start(t[:, j0:j1], hid.ap()[:, rr_, j0:j1])
                bats[rr_] = t

            # row0's k0 + W's k0 first: the PE's first real matmul only
            # needs those two 33KB chunks
            t0 = hid_pool.tile([128, KCH, T], H8, tag="ht", name="ht")
            nc.gpsimd.dma_start(t0[:, 0:1], hid.ap()[:, 0, 0:1])
            bats[0] = t0
            nc.gpsimd.dma_start(wt[:, 0:1], w_d.ap()[:, 0:1])
            nc.gpsimd.dma_start(t0[:, 1:3], hid.ap()[:, 0, 1:3])
            nc.gpsimd.dma_start(wt[:, 1:KCH], w_d.ap()[:, 1:KCH])
            nc.gpsimd.dma_start(t0[:, 3:KCH], hid.ap()[:, 0, 3:KCH])
            iota_sb = const_pool.tile([128, T], HP)
            nc.sync.dma_start(iota_sb[:], iota_d.ap()[:])
            segt = const_pool.tile([128, 2, rpc], F32)
            nc.sync.dma_start(segt[:], seg_d.ap()[:])
            gt = const_pool.tile([128, 2, rpc], F32)
            nc.sync.dma_start(gt[:], g_d.ap()[:])
            if with_bias:
                ones = const_pool.tile([1, 128], HP)
                nc.vector.memset(ones[:], 1.0)
                bsb = const_pool.tile([1, S], HP)
                nc.sync.dma_start(bsb[:], b_d.ap()[:])

            # deep head start on the gpsimd ring so the cold (throttled) PE
            # never starves while the HAM clock warms up
            for rr_ in range(1, 6):
                fetch_row(rr_)

            def emit_stage2(item):
                pr, plsb, pmbar = item
                ppair, prr = divmod(pr, 2)
                # out[w, s] = sum_c Mg[:, c, wchunk].T @ lsb[:, c, :] with Mg
                # stationary and the fp16 logits moving (N=130 stream)
                pso = pso_pool.tile([128, 2, S], F32, name="pso")
                for wc in range(2):
                    for c in range(2):
                        nc.tensor.matmul(
                            pso[:, wc, :],
                            pmbar[:, c, 128 * wc : 128 * (wc + 1)],
                            plsb[:, c, :],
                            start=(c == 0),
                            stop=(c == 1),
                        )
                # PSUM -> SBUF fp16 on ACT; DMA out every 2 rows on sync,
                # except the final pair which goes per-row to cut the
                # end-of-kernel drain
                if prr == 0:
                    obs[ppair] = o_pool.tile([128, 2, 2, S], HP, tag="ob", name="ob")
                ob = obs[ppair]
                # every 3rd output cast goes to DVE so neither ACT nor DVE
                # exceeds the 928ns/row PE budget
                if pr % 3 == 0:
                    nc.vector.tensor_copy(ob[:, prr], pso[:])
                else:
                    nc.scalar.copy(ob[:, prr], pso[:])
                last_pair = ppair == nbatch - 1
                if last_pair:
                    # per-row, on the fast gpsimd ring: shrinks the
                    # end-of-kernel drain (the sync/HWDGE ring is slow)
                    nc.gpsimd.dma_start(
                        out_d.ap()[:, pr : pr + 1], ob[:, prr : prr + 1]
                    )
                elif prr == 1:
                    nc.sync.dma_start(out_d.ap()[:, 2 * ppair : 2 * ppair + 2], ob[:])

            pending = None
            for r in range(rpc):
                if r + 6 < rpc:
                    fetch_row(r + 6)
                ht = bats.pop(r)

                # Mg[t, w] = (seg[t] == w) * g[t], fp16 in and out (2x DVE
                # throughput), t-chunked, on DVE
                mbar = m_pool.tile([128, 2, T], HP)
                for c in range(2):
                    nc.vector.tensor_scalar(
                        mbar[:, c, :],
                        iota_sb[:],
                        segt[:, c, r : r + 1],
                        gt[:, c, r : r + 1],
                        eq,
                        mult,
                    )

                # stage 1: logits for both t-chunks into one fp32 PSUM tile
                psl = psl_pool.tile([128, 2, S], F32)
                for c in range(2):
                    for k in range(KCH):
                        nc.tensor.matmul(
                            psl[:, c, :],
                            ht[:, k, 128 * c : 128 * (c + 1)],
                            wt[:, k, :],
                            start=(k == 0),
                            stop=(k == KCH - 1 and not with_bias),
                        )
                    if with_bias:
                        nc.tensor.matmul(
                            psl[:, c, :], ones[:], bsb[:], start=False, stop=True
                        )

                # stage 2 of the PREVIOUS row goes on the PE queue here so the
                # PE never waits on the DVE-produced lsb of the same row
                if pending is not None:
                    emit_stage2(pending)

                # PSUM -> SBUF fp16 in one ACT copy (DVE is busy with the
                # Mg builds; ACT runs one row behind, off the critical path)
                lsb = l_pool.tile([128, 2, S], HP)
                nc.scalar.copy(lsb[:], psl[:])
                pending = (r, lsb, mbar)
            emit_stage2(pending)

    if split_waits:
        _split_sync_waits(nc)
    return nc


def _host_prep(hidden, W, b, seg):
    """Pure layout/encoding prep (no float arithmetic on the model data
    beyond 1/count of the integer segment ids)."""
    # [core][p, r, k, t] with p the SBUF partition (= h % 128 within chunk k)
    import ml_dtypes

    h8 = np.asarray(hidden[:, 1:, :], dtype=np.float32).astype(ml_dtypes.float8_e3m4)
    h8 = h8.reshape(N_CORES, RPC, T, KCH, 128)
    hiddenT = np.ascontiguousarray(h8.transpose(0, 4, 1, 3, 2))

    seg = np.asarray(seg)
    counts = np.zeros((B, T), dtype=np.int64)
    rows = np.arange(B)[:, None]
    np.add.at(counts, (rows, seg), 1)
    g = (1.0 / np.maximum(counts, 1))[rows, seg].astype(np.float32)  # [B, T]
    segf = seg.astype(np.float32)

    # partition-major packing: [core][p, c, r] = value at (row0+r, 128c+p)
    def pack(x):
        # x: [B, T] -> [N_CORES, 128, 2, RPC]
        x4 = x.reshape(N_CORES, RPC, 2, 128)  # [core, r, c, p]
        return np.ascontiguousarray(x4.transpose(0, 3, 2, 1))

    segt = pack(segf)
    gt = pack(g)
    w16 = np.asarray(W, dtype=np.float32).astype(np.float16).reshape(KCH, 128, S)
    w_in = np.ascontiguousarray(w16.transpose(1, 0, 2))  # [128, KCH, S]
    b_in = np.ascontiguousarray(b, dtype=np.float32).astype(np.float16).reshape(1, S)
    iota16 = np.broadcast_to(
        np.arange(T, dtype=np.float16)[None, :], (128, T)
    ).copy()
    return hiddenT, w_in, b_in, segt, gt, iota16


_CACHE = {}


def kernel(hidden, W, b, seg):
    hiddenT, w_in, b_in, segt, gt, iota16 = _host_prep(hidden, W, b, seg)
    with_bias = bool(np.any(b_in != 0.0))

    key = ("prog", with_bias)
    if key not in _CACHE:
        _CACHE[key] = _build_program(with_bias=with_bias)
    nc = _CACHE[key]

    in_maps = []
    for c in range(N_CORES):
        in_maps.append(
            {
                "hiddent": hiddenT[c],
                "w": w_in,
                "bvec": b_in,
                "segt": segt[c],
                "gt": gt[c],
                "iota16": iota16,
            }
        )
    res = run_bass_kernel_spmd(nc, in_maps, core_ids=list(range(N_CORES)))
    # device layout is [w_part=128, RPC, w_chunk=2, S]; out[b, s, 128*wc + p]
    # = dev[p, r, wc, s] -> transpose to [RPC, S, wc, p] and flatten t.
    parts = []
    for c in range(N_CORES):
        dev = res.results[c]["out"]  # [128, RPC, 2, S] fp16
        parts.append(
            dev.transpose(1, 3, 2, 0).reshape(RPC, S, T).astype(np.float32)
        )
    return np.ascontiguousarray(np.concatenate(parts, axis=0))


# revision 13
# speedup vs baseline: 1.0498x; 1.0498x over previous
"""Trainium2 Bass kernel for nn_ModelIAS_53618371724066 (segment_reduce).

Computes, for each batch row b:
    logits = hidden[b, 1:, :] @ W + b_vec          # [T, S]
    merged[w, :] = mean over {t : seg[b,t] == w} of logits[t, :]   (0 if empty)
    out[b] = merged.T                               # [S, T]

Strategy (data-parallel over batch, 32 rows per core on 8 cores):
  - hidden is host-transposed to [p, row, k, t] and quantized to fp8 e3m4
    (4 mantissa bits; |h| < 15.5 so range is safe): HALVES input HBM
    traffic and measures rel_err 1.41e-2, inside the 2e-2 gate.  W stays
    fp16 (fp8 W pushes the error past the gate).
  - Warm steady state is PE-stream-bound: 16 matmuls/row x ~58ns
    (N=130 columns at 2.4GHz, LDWEIGHTS hidden by FWL+shadow plane)
    = 928ns/row.  Everything else is scheduled to stay under that:
      DVE: 2x Mg tensor_scalar (fp16 iota input) + lsb PSUM->SBUF copy
      ACT: output PSUM->SBUF fp16 cast only
      GpSimd: hidden DMA issue, batched 2 rows per descriptor set
      Sync: W/seg/g/iota constants + output DMAs
  - The HAM clock gate needs ~3us of CONTINUOUS PE busy to unthrottle
    0.65->1.2->2.4GHz; any idle gap resets it.  So: (a) a few junk
    matmuls on a memset tile bridge the initial DMA wait, (b) hidden is
    prefetched 3 batches (6 rows) deep so the PE never starves, (c) the
    first fetch is split so row0's k0 chunk + W's k0 chunk land first
    (on the otherwise-idle sync/HWDGE ring) and the PE starts ~1us
    earlier.
  - Stage 1 (PE): logits[t_chunk, s] accumulated over 6 k-chunks into one
    PSUM tile [128, 2, S] fp32; bias folded as rank-1 matmul if nonzero.
  - Stage 2 (PE): out[w, s] = sum_c Mg[:, c, wchunk].T @ lsb[:, c, :],
    Mg stationary (built by DVE as (seg==w)*g in fp16), fp16 logits
    moving; emitted AFTER stage 1 of the next row (1-row software
    pipeline) so the in-order PE never waits on the lsb copy.
  - Output: ACT casts PSUM->SBUF fp16 per row into a 2-row tile; DMA out
    every 2 rows on the sync ring (the last pair goes per-row to shrink
    the end-of-kernel drain).  Host reassembles [w, s] -> [s, t].
  - Per-instruction sem-waits are legalized for the pinned walrus by
    _split_sync_waits.
"""

import numpy as np

import concourse.bass as bass
import concourse.tile as tile
from concourse import mybir
from concourse.bass_utils import run_bass_kernel_spmd

B, T, H, S = 256, 256, 768, 130
N_CORES = 8
RPC = B // N_CORES  # rows per core
KCH = H // 128  # k chunks of the hidden dim
F32 = mybir.dt.float32
HP = mybir.dt.float16
H8 = mybir.dt.float8e3  # e3m4: 4 mantissa bits, covers |h|<~15.5


def _split_sync_waits(nc):
    """The pinned walrus build rejects instructions carrying more than one
    sync-wait command ("Too many sync wait commands", setupSyncWait).  Keep
    one wait per instruction and hoist the rest onto NoOps inserted just
    before it on the same engine (same semantics: all waits still execute
    before the instruction, in stream order)."""
    for f in nc.m.functions:
        for blk in f.blocks:
            il = blk.instructions
            i = 0
            while i < len(il):
                inst = il[i]
                si = inst.sync_info
                if si is not None and si.on_wait and len(si.on_wait) >= 2:
                    waits = list(si.on_wait)
                    keep = [waits.pop()]
                    pos = i
                    for j, w in enumerate(waits):
                        nop = mybir.InstNoOp(name=f"{inst.name}_ws{j}", ins=[], outs=[])
                        nop.engine = inst.engine
                        nop.sync_info = mybir.SyncInfo(on_wait=[w], on_update=[])
                        il.insert(pos, nop)
                        pos += 1
                        i += 1
                    inst.sync_info = mybir.SyncInfo(
                        on_wait=keep, on_update=list(si.on_update)
                    )
                i += 1


def _build_program(rpc=RPC, with_bias=False, hid_bufs=10, n_junk=6, split_waits=True):
    nc = bass.Bass("TRN2", target_bir_lowering=False, debug=False)

    hid = nc.dram_tensor("hiddent", [128, rpc, KCH, T], H8, kind="ExternalInput")
    w_d = nc.dram_tensor("w", [128, KCH, S], HP, kind="ExternalInput")
    b_d = nc.dram_tensor("bvec", [1, S], HP, kind="ExternalInput")
    seg_d = nc.dram_tensor("segt", [128, 2, rpc], F32, kind="ExternalInput")
    g_d = nc.dram_tensor("gt", [128, 2, rpc], F32, kind="ExternalInput")
    iota_d = nc.dram_tensor("iota16", [128, T], HP, kind="ExternalInput")
    # [w_partition, row, w_chunk, s] fp16; host reassembles to [B, S, T]
    out_d = nc.dram_tensor("out", [128, rpc, 2, S], HP, kind="ExternalOutput")

    eq = mybir.AluOpType.is_equal
    mult = mybir.AluOpType.mult
    assert rpc % 2 == 0
    nbatch = rpc // 2
    with tile.TileContext(nc) as tc:
        with (
            tc.tile_pool(name="const", bufs=1) as const_pool,
            tc.tile_pool(name="hid", bufs=hid_bufs) as hid_pool,
            tc.tile_pool(name="mbar", bufs=3) as m_pool,
            tc.tile_pool(name="lsb", bufs=3) as l_pool,
            tc.tile_pool(name="osb", bufs=3) as o_pool,
            tc.tile_pool(name="psl", bufs=3, space=bass.MemorySpace.PSUM) as psl_pool,
            tc.tile_pool(name="pso", bufs=4, space=bass.MemorySpace.PSUM) as pso_pool,
            tc.tile_pool(name="psj", bufs=1, space=bass.MemorySpace.PSUM) as psj_pool,
        ):
            # --- junk warm-up matmuls: keep the PE continuously busy from
            # t~0 so the HAM clock ramp (needs 3us of uninterrupted busy)
            # starts before row 0's data lands.  Inputs are memset zeros;
            # output goes to a scratch PSUM bank.
            bats = {}
            obs = {}
            wt = const_pool.tile([128, KCH, S], HP)
            if n_junk:
                jw = const_pool.tile([128, 128], HP)
                nc.vector.memset(jw[:], 0.0)
                jm = const_pool.tile([128, 256], HP)
                nc.vector.memset(jm[:], 0.0)
                psj = psj_pool.tile([128, 256], F32)
                for _ in range(n_junk):
                    nc.tensor.matmul(psj[:], jw[:], jm[:], start=True, stop=True)

            # --- head start.  The HWDGE rings (sync/scalar) move bytes
            # ~10x slower than the gpsimd SWDGE ring for these small-run
            # shapes, so hidden AND W ride SWDGE; only the tiny constants
            # (iota/seg/g, needed ~3us in) go on the sync ring whose issue
            # runs in parallel with gpsimd's. ---
            def fetch_row(rr_, chunks=((0, KCH),)):
                t = hid_pool.tile([128, KCH, T], H8, tag="ht", name="ht")
                for j0, j1 in chunks:
                    nc.gpsimd.dma_start(t[:, j0:j1], hid.ap()[:, rr_, j0:j1])
                bats[rr_] = t

            # row0's k0 + W's k0 first: the PE's first real matmul only
            # needs those two 33KB chunks
            t0 = hid_pool.tile([128, KCH, T], H8, tag="ht", name="ht")
            nc.gpsimd.dma_start(t0[:, 0:1], hid.ap()[:, 0, 0:1])
            bats[0] = t0
            nc.gpsimd.dma_start(wt[:, 0:1], w_d.ap()[:, 0:1])
            nc.gpsimd.dma_start(t0[:, 1:3], hid.ap()[:, 0, 1:3])
            nc.gpsimd.dma_start(wt[:, 1:KCH], w_d.ap()[:, 1:KCH])
            nc.gpsimd.dma_start(t0[:, 3:KCH], hid.ap()[:, 0, 3:KCH])
            iota_sb = const_pool.tile([128, T], HP)
            nc.sync.dma_start(iota_sb[:], iota_d.ap()[:])
            segt = const_pool.tile([128, 2, rpc], F32)
            nc.sync.dma_start(segt[:], seg_d.ap()[:])
            gt = const_pool.tile([128, 2, rpc], F32)
            nc.sync.dma_start(gt[:], g_d.ap()[:])
            if with_bias:
                ones = const_pool.tile([1, 128], HP)
                nc.vector.memset(ones[:], 1.0)
                bsb = const_pool.tile([1, S], HP)
                nc.sync.dma_start(bsb[:], b_d.ap()[:])

            # deep head start on the gpsimd ring so the cold (throttled) PE
            # never starves while the HAM clock warms up
            for rr_ in range(1, 6):
                fetch_row(rr_)

            def emit_stage2(item):
                pr, plsb, pmbar = item
                ppair, prr = divmod(pr, 2)
                # out[w, s] = sum_c Mg[:, c, wchunk].T @ lsb[:, c, :] with Mg
                # stationary and the fp16 logits moving (N=130 stream)
                pso = pso_pool.tile([128, 2, S], F32, name="pso")
                for wc in range(2):
                    for c in range(2):
                        nc.tensor.matmul(
                            pso[:, wc, :],
                            pmbar[:, c, 128 * wc : 128 * (wc + 1)],
                            plsb[:, c, :],
                            start=(c == 0),
                            stop=(c == 1),
                        )
                # PSUM -> SBUF fp16 on ACT; DMA out every 2 rows on sync,
                # except the final pair which goes per-row to cut the
                # end-of-kernel drain
                if prr == 0:
                    obs[ppair] = o_pool.tile([128, 2, 2, S], HP, tag="ob", name="ob")
                ob = obs[ppair]
                nc.scalar.copy(ob[:, prr], pso[:])
                if prr == 1:
                    nc.sync.dma_start(out_d.ap()[:, 2 * ppair : 2 * ppair + 2], ob[:])

            pending = None
            for r in range(rpc):
                if r + 6 < rpc:
                    fetch_row(r + 6)
                ht = bats.pop(r)

                # Mg[t, w] = (seg[t] == w) * g[t], fp16 in and out (2x DVE
                # throughput), t-chunked, on DVE
                mbar = m_pool.tile([128, 2, T], HP)
                for c in range(2):
                    nc.vector.tensor_scalar(
                        mbar[:, c, :],
                        iota_sb[:],
                        segt[:, c, r : r + 1],
                        gt[:, c, r : r + 1],
                        eq,
                        mult,
                    )

                # stage 1: logits for both t-chunks into one fp32 PSUM tile
                psl = psl_pool.tile([128, 2, S], F32)
                for c in range(2):
                    for k in range(KCH):
                        nc.tensor.matmul(
                            psl[:, c, :],
                            ht[:, k, 128 * c : 128 * (c + 1)],
                            wt[:, k, :],
                            start=(k == 0),
                            stop=(k == KCH - 1 and not with_bias),
                        )
                    if with_bias:
                        nc.tensor.matmul(
                            psl[:, c, :], ones[:], bsb[:], start=False, stop=True
                        )

                # stage 2 of the PREVIOUS row goes on the PE queue here so the
                # PE never waits on the DVE-produced lsb of the same row
                if pending is not None:
                    emit_stage2(pending)

                # PSUM -> SBUF fp16 in one ACT copy (DVE is busy with the
                # Mg builds; ACT runs one row behind, off the critical path)
                lsb = l_pool.tile([128, 2, S], HP)
                nc.scalar.copy(lsb[:], psl[:])
                pending = (r, lsb, mbar)
            emit_stage2(pending)

    if split_waits:
        _split_sync_waits(nc)
    return nc


def _host_prep(hidden, W, b, seg):
    """Pure layout/encoding prep (no float arithmetic on the model data
    beyond 1/count of the integer segment ids)."""
    # [core][p, r, k, t] with p the SBUF partition (= h % 128 within chunk k)
    import ml_dtypes

    h8 = np.asarray(hidden[:, 1:, :], dtype=np.float32).astype(ml_dtypes.float8_e3m4)
    h8 = h8.reshape(N_CORES, RPC, T, KCH, 128)
    hiddenT = np.ascontiguousarray(h8.transpose(0, 4, 1, 3, 2))

    seg = np.asarray(seg)
    counts = np.zeros((B, T), dtype=np.int64)
    rows = np.arange(B)[:, None]
    np.add.at(counts, (rows, seg), 1)
    g = (1.0 / np.maximum(counts, 1))[rows, seg].astype(np.float32)  # [B, T]
    segf = seg.astype(np.float32)

    # partition-major packing: [core][p, c, r] = value at (row0+r, 128c+p)
    def pack(x):
        # x: [B, T] -> [N_CORES, 128, 2, RPC]
        x4 = x.reshape(N_CORES, RPC, 2, 128)  # [core, r, c, p]
        return np.ascontiguousarray(x4.transpose(0, 3, 2, 1))

    segt = pack(segf)
    gt = pack(g)
    w16 = np.asarray(W, dtype=np.float32).astype(np.float16).reshape(KCH, 128, S)
    w_in = np.ascontiguousarray(w16.transpose(1, 0, 2))  # [128, KCH, S]
    b_in = np.ascontiguousarray(b, dtype=np.float32).astype(np.float16).reshape(1, S)
    iota16 = np.broadcast_to(
        np.arange(T, dtype=np.float16)[None, :], (128, T)
    ).copy()
    return hiddenT, w_in, b_in, segt, gt, iota16


_CACHE = {}


def kernel(hidden, W, b, seg):
    hiddenT, w_in, b_in, segt, gt, iota16 = _host_prep(hidden, W, b, seg)
    with_bias = bool(np.any(b_in != 0.0))

    key = ("prog", with_bias)
    if key not in _CACHE:
        _CACHE[key] = _build_program(with_bias=with_bias)
    nc = _CACHE[key]

    in_maps = []
    for c in range(N_CORES):
        in_maps.append(
            {
                "hiddent": hiddenT[c],
                "w": w_in,
                "bvec": b_in,
                "segt": segt[c],
                "gt": gt[c],
                "iota16": iota16,
            }
        )
    res = run_bass_kernel_spmd(nc, in_maps, core_ids=list(range(N_CORES)))
    # device layout is [w_part=128, RPC, w_chunk=2, S]; out[b, s, 128*wc + p]
    # = dev[p, r, wc, s] -> transpose to [RPC, S, wc, p] and flatten t.
    parts = []
    for c in range(N_CORES):
        dev = res.results[c]["out"]  # [128, RPC, 2, S] fp16
        parts.append(
            dev.transpose(1, 3, 2, 0).reshape(RPC, S, T).astype(np.float32)
        )
    return np.ascontiguousarray(np.concatenate(parts, axis=0))


# revision 14
# speedup vs baseline: 1.0961x; 1.0441x over previous
"""Trainium2 Bass kernel for nn_ModelIAS_53618371724066 (segment_reduce).

Computes, for each batch row b:
    logits = hidden[b, 1:, :] @ W + b_vec          # [T, S]
    merged[w, :] = mean over {t : seg[b,t] == w} of logits[t, :]   (0 if empty)
    out[b] = merged.T                               # [S, T]

Strategy (data-parallel over batch, 32 rows per core on 8 cores):
  - Host prep is layout/precision only: hidden is transposed to a
    partition-major [p, row, k, t] layout and quantized to fp8 e3m4 (4
    mantissa bits; |h| < 15.5 so range is safe) -- this HALVES the input
    HBM traffic vs fp16 and measures rel_err 1.41e-2 on hardware, inside
    the 2e-2 gate with margin; the PE runs mixed fp8e3-stationary x
    fp16-moving matmuls with fp32 PSUM accumulation.  W stays fp16 (fp8 W
    would push the error past the gate).
  - The mean-weighted segment matrix Mg[t, w] = g[t] * (seg[t] == w) with
    g[t] = 1/count[seg[t]] is built on-chip in fp16 with one dual-op DVE
    tensor_scalar per t-chunk (is_equal then mult, both per-partition
    scalars), so the mean normalization costs nothing extra.
  - Stage 1 (PE): logits[t_chunk, s] = sum_k hiddenT[k-chunk].T @ W[k-chunk]
    accumulated in fp32 PSUM; both t-chunks land in ONE PSUM tile
    [128, 2, S] so the PSUM->SBUF fp16 evacuation is a single ACT copy.
    Bias is folded in as a rank-1 matmul when b != 0.
  - Stage 2 (PE): out[w, s] = sum_c Mg[:, c, wchunk].T @ lsb[:, c, :] with
    Mg STATIONARY and the fp16 logits moving (N=130 stream, not 256) —
    this is ~2x fewer PE streaming cycles than the lsb-stationary
    formulation and has no wasted [128,2]-stationary tail matmul.
  - Both w-chunks of stage 2 accumulate into one PSUM tile [128, 2, S];
    ACT casts it to fp16 in a single copy (DMA cannot read PSUM, and the
    Pool/gpsimd engine cannot touch PSUM either), and output DMAs go out
    2 rows at a time (fp16, half the bytes of fp32) with 1 KB contiguous
    runs per partition.  Host reassembles [w, s] -> [s, t].
  - Stage 2 of row r-1 is emitted on the PE queue AFTER stage 1 of row r
    (one-row software pipeline) so the in-order PE never stalls on the
    ACT-produced lsb of the same row.  Per-row engine budget: PE ~0.94us,
    ACT ~0.95us (lsb + output cast), DVE ~0.82us (2x Mg build).
  - Hidden streams on the GpSimd ring in 1-row DMAs (SWDGE; moving it to
    the SP HWDGE ring oversubscribes that sequencer and measures slower)
    while outputs and constants use the SP ring; per-instruction sem-waits are legalized for
    the pinned walrus by _split_sync_waits.
"""

import numpy as np

import concourse.bass as bass
import concourse.tile as tile
from concourse import mybir
from concourse.bass_utils import run_bass_kernel_spmd

B, T, H, S = 256, 256, 768, 130
N_CORES = 8
RPC = B // N_CORES  # rows per core
KCH = H // 128  # k chunks of the hidden dim
F32 = mybir.dt.float32
HP = mybir.dt.float16
H8 = mybir.dt.float8e3  # e3m4: 4 mantissa bits, covers |h|<~15.5


def _split_sync_waits(nc):
    """The pinned walrus build rejects instructions carrying more than one
    sync-wait command ("Too many sync wait commands", setupSyncWait).  Keep
    one wait per instruction and hoist the rest onto NoOps inserted just
    before it on the same engine (same semantics: all waits still execute
    before the instruction, in stream order)."""
    for f in nc.m.functions:
        for blk in f.blocks:
            il = blk.instructions
            i = 0
            while i < len(il):
                inst = il[i]
                si = inst.sync_info
                if si is not None and si.on_wait and len(si.on_wait) >= 2:
                    waits = list(si.on_wait)
                    keep = [waits.pop()]
                    pos = i
                    for j, w in enumerate(waits):
                        nop = mybir.InstNoOp(name=f"{inst.name}_ws{j}", ins=[], outs=[])
                        nop.engine = inst.engine
                        nop.sync_info = mybir.SyncInfo(on_wait=[w], on_update=[])
                        il.insert(pos, nop)
                        pos += 1
                        i += 1
                    inst.sync_info = mybir.SyncInfo(
                        on_wait=keep, on_update=list(si.on_update)
                    )
                i += 1


def _build_program(rpc=RPC, with_bias=False, hid_bufs=10, split_waits=True):
    nc = bass.Bass("TRN2", target_bir_lowering=False, debug=False)

    hid = nc.dram_tensor("hiddent", [128, rpc, KCH, T], H8, kind="ExternalInput")
    w_d = nc.dram_tensor("w", [128, KCH, S], HP, kind="ExternalInput")
    b_d = nc.dram_tensor("bvec", [1, S], HP, kind="ExternalInput")
    seg_d = nc.dram_tensor("segt", [128, 2, rpc], F32, kind="ExternalInput")
    g_d = nc.dram_tensor("gt", [128, 2, rpc], F32, kind="ExternalInput")
    # [w_partition, row, w_chunk, s] fp16; host reassembles to [B, S, T]
    out_d = nc.dram_tensor("out", [128, rpc, 2, S], HP, kind="ExternalOutput")

    eq = mybir.AluOpType.is_equal
    mult = mybir.AluOpType.mult
    assert rpc % 2 == 0
    with tile.TileContext(nc) as tc:
        with (
            tc.tile_pool(name="const", bufs=1) as const_pool,
            tc.tile_pool(name="hid", bufs=hid_bufs) as hid_pool,
            tc.tile_pool(name="mbar", bufs=3) as m_pool,
            tc.tile_pool(name="lsb", bufs=3) as l_pool,
            tc.tile_pool(name="osb", bufs=4) as o_pool,
            tc.tile_pool(name="psl", bufs=3, space=bass.MemorySpace.PSUM) as psl_pool,
            tc.tile_pool(name="pso", bufs=5, space=bass.MemorySpace.PSUM) as pso_pool,
        ):
            # --- constants; hidden rows stream in 1-row fp8 DMAs on the
            # gpsimd ring (~0.2MB each), prefetched 2 rows ahead ---
            hts = {}
            obs = {}

            def fetch_row(rr_, chunks=((0, KCH),)):
                t = hid_pool.tile([128, KCH, T], H8, tag="ht", name="ht")
                for j0, j1 in chunks:
                    nc.gpsimd.dma_start(t[:, j0:j1], hid.ap()[:, rr_, j0:j1])
                hts[rr_] = t

            # row 0 lands k-chunk 0 first so the PE starts ~1.5us earlier
            # (the tile deps are per-DMA, so matmul k=0 only waits chunk 0)
            fetch_row(0, chunks=((0, 1), (1, 3), (3, KCH)))
            wt = const_pool.tile([128, KCH, S], HP)
            nc.sync.dma_start(wt[:], w_d.ap()[:])
            segt = const_pool.tile([128, 2, rpc], F32)
            nc.sync.dma_start(segt[:], seg_d.ap()[:])
            gt = const_pool.tile([128, 2, rpc], F32)
            nc.sync.dma_start(gt[:], g_d.ap()[:])
            iota_i = const_pool.tile([128, T], mybir.dt.int32)
            nc.gpsimd.iota(iota_i[:], pattern=[[1, T]], base=0, channel_multiplier=0)
            iota_f = const_pool.tile([128, T], F32)
            nc.vector.tensor_copy(iota_f[:], iota_i[:])
            if with_bias:
                ones = const_pool.tile([1, 128], HP)
                nc.vector.memset(ones[:], 1.0)
                bsb = const_pool.tile([1, S], HP)
                nc.sync.dma_start(bsb[:], b_d.ap()[:])

            fetch_row(1)

            def emit_stage2(item):
                pr, plsb, pmbar = item
                ppair, prr = divmod(pr, 2)
                # out[w, s] = sum_c Mg[:, c, wchunk].T @ lsb[:, c, :] with Mg
                # stationary and the fp16 logits moving (N=130 stream)
                pso = pso_pool.tile([128, 2, S], F32, name="pso")
                for wc in range(2):
                    for c in range(2):
                        nc.tensor.matmul(
                            pso[:, wc, :],
                            pmbar[:, c, 128 * wc : 128 * (wc + 1)],
                            plsb[:, c, :],
                            start=(c == 0),
                            stop=(c == 1),
                        )
                # PSUM -> SBUF fp16 on ACT; DMA out every 2 rows on SP
                if prr == 0:
                    obs[ppair] = o_pool.tile([128, 2, 2, S], HP, tag="ob", name="ob")
                ob = obs[ppair]
                nc.scalar.copy(ob[:, prr], pso[:])
                if prr == 1:
                    nc.sync.dma_start(out_d.ap()[:, 2 * ppair : 2 * ppair + 2], ob[:])

            pending = None
            for r in range(rpc):
                if r + 2 < rpc:
                    fetch_row(r + 2)
                ht = hts.pop(r)

                # Mg[t, w] = (seg[t] == w) * g[t], fp16, t-chunked, on DVE
                # (gpsimd tensor_scalar is a ~4us DSP program -- never use it)
                mbar = m_pool.tile([128, 2, T], HP)
                for c in range(2):
                    nc.vector.tensor_scalar(
                        mbar[:, c, :],
                        iota_f[:],
                        segt[:, c, r : r + 1],
                        gt[:, c, r : r + 1],
                        eq,
                        mult,
                    )

                # stage 1: logits for both t-chunks into one fp32 PSUM tile
                psl = psl_pool.tile([128, 2, S], F32)
                for c in range(2):
                    for k in range(KCH):
                        nc.tensor.matmul(
                            psl[:, c, :],
                            ht[:, k, 128 * c : 128 * (c + 1)],
                            wt[:, k, :],
                            start=(k == 0),
                            stop=(k == KCH - 1 and not with_bias),
                        )
                    if with_bias:
                        nc.tensor.matmul(
                            psl[:, c, :], ones[:], bsb[:], start=False, stop=True
                        )

                # stage 2 of the PREVIOUS row goes on the PE queue here so the
                # PE never waits on the ACT-produced lsb of the same row
                if pending is not None:
                    emit_stage2(pending)

                # PSUM -> SBUF fp16 in one ACT copy (g lives in Mg, not here)
                lsb = l_pool.tile([128, 2, S], HP)
                nc.scalar.copy(lsb[:], psl[:])
                pending = (r, lsb, mbar)
            emit_stage2(pending)

    if split_waits:
        _split_sync_waits(nc)
    return nc


def _host_prep(hidden, W, b, seg):
    """Pure layout/encoding prep (no float arithmetic on the model data
    beyond 1/count of the integer segment ids)."""
    # [core][p, r, k, t] with p the SBUF partition (= h % 128 within chunk k)
    import ml_dtypes

    h8 = np.asarray(hidden[:, 1:, :], dtype=np.float32).astype(ml_dtypes.float8_e3m4)
    h8 = h8.reshape(N_CORES, RPC, T, KCH, 128)
    hiddenT = np.ascontiguousarray(h8.transpose(0, 4, 1, 3, 2))

    seg = np.asarray(seg)
    counts = np.zeros((B, T), dtype=np.int64)
    rows = np.arange(B)[:, None]
    np.add.at(counts, (rows, seg), 1)
    g = (1.0 / np.maximum(counts, 1))[rows, seg].astype(np.float32)  # [B, T]
    segf = seg.astype(np.float32)

    # partition-major packing: [core][p, c, r] = value at (row0+r, 128c+p)
    def pack(x):
        # x: [B, T] -> [N_CORES, 128, 2, RPC]
        x4 = x.reshape(N_CORES, RPC, 2, 128)  # [core, r, c, p]
        return np.ascontiguousarray(x4.transpose(0, 3, 2, 1))

    segt = pack(segf)
    gt = pack(g)
    w16 = np.asarray(W, dtype=np.float32).astype(np.float16).reshape(KCH, 128, S)
    w_in = np.ascontiguousarray(w16.transpose(1, 0, 2))  # [128, KCH, S]
    b_in = np.ascontiguousarray(b, dtype=np.float32).astype(np.float16).reshape(1, S)
    return hiddenT, w_in, b_in, segt, gt


_CACHE = {}


def kernel(hidden, W, b, seg):
    hiddenT, w_in, b_in, segt, gt = _host_prep(hidden, W, b, seg)
    with_bias = bool(np.any(b_in != 0.0))

    key = ("prog", with_bias)
    if key not in _CACHE:
        _CACHE[key] = _build_program(with_bias=with_bias)
    nc = _CACHE[key]

    in_maps = []
    for c in range(N_CORES):
        in_maps.append(
            {
                "hiddent": hiddenT[c],
                "w": w_in,
                "bvec": b_in,
                "segt": segt[c],
                "gt": gt[c],
            }
        )
    res = run_bass_kernel_spmd(nc, in_maps, core_ids=list(range(N_CORES)))
    # device layout is [w_part=128, RPC, w_chunk=2, S]; out[b, s, 128*wc + p]
    # = dev[p, r, wc, s] -> transpose to [RPC, S, wc, p] and flatten t.
    parts = []
    for c in range(N_CORES):
        dev = res.results[c]["out"]  # [128, RPC, 2, S] fp16
        parts.append(
            dev.transpose(1, 3, 2, 0).reshape(RPC, S, T).astype(np.float32)
        )
    return np.ascontiguousarray(np.concatenate(parts, axis=0))



# revision 16
# speedup vs baseline: 1.1013x; 1.0048x over previous
"""Trainium2 Bass kernel for nn_ModelIAS_53618371724066 (segment_reduce).

Computes, for each batch row b:
    logits = hidden[b, 1:, :] @ W + b_vec          # [T, S]
    merged[w, :] = mean over {t : seg[b,t] == w} of logits[t, :]   (0 if empty)
    out[b] = merged.T                               # [S, T]

Strategy (data-parallel over batch, 32 rows per core on 8 cores):
  - Host prep is layout/precision only: hidden is transposed to a
    partition-major [p, row, k, t] layout and quantized to fp8 e3m4 (4
    mantissa bits; |h| < 15.5 so range is safe) -- this HALVES the input
    HBM traffic vs fp16 and measures rel_err 1.41e-2 on hardware, inside
    the 2e-2 gate with margin; the PE runs mixed fp8e3-stationary x
    fp16-moving matmuls with fp32 PSUM accumulation.  W stays fp16 (fp8 W
    would push the error past the gate).
  - The mean-weighted segment matrix Mg[t, w] = g[t] * (seg[t] == w) with
    g[t] = 1/count[seg[t]] is built on-chip in fp16 with one dual-op DVE
    tensor_scalar per t-chunk (is_equal then mult, both per-partition
    scalars), so the mean normalization costs nothing extra.
  - Stage 1 (PE): logits[t_chunk, s] = sum_k hiddenT[k-chunk].T @ W[k-chunk]
    accumulated in fp32 PSUM; both t-chunks land in ONE PSUM tile
    [128, 2, S] so the PSUM->SBUF fp16 evacuation is a single ACT copy.
    Bias is folded in as a rank-1 matmul when b != 0.
  - Stage 2 (PE): out[w, s] = sum_c Mg[:, c, wchunk].T @ lsb[:, c, :] with
    Mg STATIONARY and the fp16 logits moving (N=130 stream, not 256) —
    this is ~2x fewer PE streaming cycles than the lsb-stationary
    formulation and has no wasted [128,2]-stationary tail matmul.
  - Both w-chunks of stage 2 accumulate into one PSUM tile [128, 2, S];
    ACT casts it to fp16 in a single copy (DMA cannot read PSUM, and the
    Pool/gpsimd engine cannot touch PSUM either), and output DMAs go out
    2 rows at a time (fp16, half the bytes of fp32) with 1 KB contiguous
    runs per partition.  Host reassembles [w, s] -> [s, t].
  - Stage 2 of row r-1 is emitted on the PE queue AFTER stage 1 of row r
    (one-row software pipeline) so the in-order PE never stalls on the
    ACT-produced lsb of the same row.  Per-row engine budget: PE ~0.94us,
    ACT ~0.95us (lsb + output cast), DVE ~0.82us (2x Mg build).
  - Hidden streams on the GpSimd ring in 1-row DMAs (SWDGE; moving it to
    the SP HWDGE ring oversubscribes that sequencer and measures slower)
    while outputs and constants use the SP ring; per-instruction sem-waits are legalized for
    the pinned walrus by _split_sync_waits.
"""

import numpy as np

import concourse.bass as bass
import concourse.tile as tile
from concourse import mybir
from concourse.bass_utils import run_bass_kernel_spmd

B, T, H, S = 256, 256, 768, 130
N_CORES = 8
RPC = B // N_CORES  # rows per core
KCH = H // 128  # k chunks of the hidden dim
F32 = mybir.dt.float32
HP = mybir.dt.float16
H8 = mybir.dt.float8e3  # e3m4: 4 mantissa bits, covers |h|<~15.5


def _split_sync_waits(nc):
    """The pinned walrus build rejects instructions carrying more than one
    sync-wait command ("Too many sync wait commands", setupSyncWait).  Keep
    one wait per instruction and hoist the rest onto NoOps inserted just
    before it on the same engine (same semantics: all waits still execute
    before the instruction, in stream order)."""
    for f in nc.m.functions:
        for blk in f.blocks:
            il = blk.instructions
            i = 0
            while i < len(il):
                inst = il[i]
                si = inst.sync_info
                if si is not None and si.on_wait and len(si.on_wait) >= 2:
                    waits = list(si.on_wait)
                    keep = [waits.pop()]
                    pos = i
                    for j, w in enumerate(waits):
                        nop = mybir.InstNoOp(name=f"{inst.name}_ws{j}", ins=[], outs=[])
                        nop.engine = inst.engine
                        nop.sync_info = mybir.SyncInfo(on_wait=[w], on_update=[])
                        il.insert(pos, nop)
                        pos += 1
                        i += 1
                    inst.sync_info = mybir.SyncInfo(
                        on_wait=keep, on_update=list(si.on_update)
                    )
                i += 1


def _build_program(rpc=RPC, with_bias=False, hid_bufs=10, split_waits=True):
    nc = bass.Bass("TRN2", target_bir_lowering=False, debug=False)

    hid = nc.dram_tensor("hiddent", [128, rpc, KCH, T], H8, kind="ExternalInput")
    w_d = nc.dram_tensor("w", [128, KCH, S], HP, kind="ExternalInput")
    b_d = nc.dram_tensor("bvec", [1, S], HP, kind="ExternalInput")
    seg_d = nc.dram_tensor("segt", [128, 2, rpc], F32, kind="ExternalInput")
    g_d = nc.dram_tensor("gt", [128, 2, rpc], F32, kind="ExternalInput")
    # [w_partition, row, w_chunk, s] fp16; host reassembles to [B, S, T]
    out_d = nc.dram_tensor("out", [128, rpc, 2, S], HP, kind="ExternalOutput")

    eq = mybir.AluOpType.is_equal
    mult = mybir.AluOpType.mult
    assert rpc % 2 == 0
    with tile.TileContext(nc) as tc:
        with (
            tc.tile_pool(name="const", bufs=1) as const_pool,
            tc.tile_pool(name="hid", bufs=hid_bufs) as hid_pool,
            tc.tile_pool(name="mbar", bufs=3) as m_pool,
            tc.tile_pool(name="lsb", bufs=3) as l_pool,
            tc.tile_pool(name="osb", bufs=4) as o_pool,
            tc.tile_pool(name="psl", bufs=3, space=bass.MemorySpace.PSUM) as psl_pool,
            tc.tile_pool(name="pso", bufs=4, space=bass.MemorySpace.PSUM) as pso_pool,
            tc.tile_pool(name="psj", bufs=1, space=bass.MemorySpace.PSUM) as psj_pool,
        ):
            # --- constants; hidden rows stream in 1-row fp8 DMAs on the
            # gpsimd ring (~0.2MB each), prefetched 2 rows ahead ---
            hts = {}
            obs = {}
            wt = const_pool.tile([128, KCH, S], HP)

            # junk warm-up matmuls on memset zeros: keep the PE busy from
            # t~0 so the HAM clock ramp (3us of continuous busy needed for
            # full 2.4GHz) completes while row 0's data is still in flight
            jw = const_pool.tile([128, 128], HP)
            nc.vector.memset(jw[:], 0.0)
            jm = const_pool.tile([128, 256], HP)
            nc.vector.memset(jm[:], 0.0)
            psj = psj_pool.tile([128, 256], F32)
            for _ in range(5):
                nc.tensor.matmul(psj[:], jw[:], jm[:], start=True, stop=True)

            def fetch_row(rr_, chunks=((0, KCH),)):
                t = hid_pool.tile([128, KCH, T], H8, tag="ht", name="ht")
                for j0, j1 in chunks:
                    nc.gpsimd.dma_start(t[:, j0:j1], hid.ap()[:, rr_, j0:j1])
                hts[rr_] = t

            # row0's k0 + W's k0 chunks (33KB each) ride the sync ring ahead
            # of everything else: the PE's first real matmul needs only these
            # two, and the sync ring's latency beats the SWDGE ring's ~1.7us
            # post-issue lag.  Remaining chunks stream on gpsimd as usual.
            t0 = hid_pool.tile([128, KCH, T], H8, tag="ht", name="ht")
            nc.sync.dma_start(t0[:, 0:1], hid.ap()[:, 0, 0:1])
            hts[0] = t0
            nc.sync.dma_start(wt[:, 0:1], w_d.ap()[:, 0:1])
            nc.gpsimd.dma_start(t0[:, 1:3], hid.ap()[:, 0, 1:3])
            nc.gpsimd.dma_start(t0[:, 3:KCH], hid.ap()[:, 0, 3:KCH])
            nc.sync.dma_start(wt[:, 1:KCH], w_d.ap()[:, 1:KCH])
            segt = const_pool.tile([128, 2, rpc], F32)
            nc.sync.dma_start(segt[:], seg_d.ap()[:])
            gt = const_pool.tile([128, 2, rpc], F32)
            nc.sync.dma_start(gt[:], g_d.ap()[:])
            iota_i = const_pool.tile([128, T], mybir.dt.int32)
            nc.gpsimd.iota(iota_i[:], pattern=[[1, T]], base=0, channel_multiplier=0)
            iota_f = const_pool.tile([128, T], HP)
            nc.vector.tensor_copy(iota_f[:], iota_i[:])
            if with_bias:
                ones = const_pool.tile([1, 128], HP)
                nc.vector.memset(ones[:], 1.0)
                bsb = const_pool.tile([1, S], HP)
                nc.sync.dma_start(bsb[:], b_d.ap()[:])

            fetch_row(1)

            def emit_stage2(item):
                pr, plsb, pmbar = item
                ppair, prr = divmod(pr, 2)
                # out[w, s] = sum_c Mg[:, c, wchunk].T @ lsb[:, c, :] with Mg
                # stationary and the fp16 logits moving (N=130 stream)
                pso = pso_pool.tile([128, 2, S], F32, name="pso")
                for wc in range(2):
                    for c in range(2):
                        nc.tensor.matmul(
                            pso[:, wc, :],
                            pmbar[:, c, 128 * wc : 128 * (wc + 1)],
                            plsb[:, c, :],
                            start=(c == 0),
                            stop=(c == 1),
                        )
                # PSUM -> SBUF fp16 on ACT; DMA out every 2 rows on SP
                if prr == 0:
                    obs[ppair] = o_pool.tile([128, 2, 2, S], HP, tag="ob", name="ob")
                ob = obs[ppair]
                nc.scalar.copy(ob[:, prr], pso[:])
                if ppair >= rpc // 2 - 2:
                    # final pairs go per-row so the end-of-kernel drain only
                    # waits on one small transfer
                    nc.sync.dma_start(
                        out_d.ap()[:, pr : pr + 1], ob[:, prr : prr + 1]
                    )
                elif prr == 1:
                    nc.sync.dma_start(out_d.ap()[:, 2 * ppair : 2 * ppair + 2], ob[:])

            pending = None
            for r in range(rpc):
                if r + 2 < rpc:
                    fetch_row(r + 2)
                ht = hts.pop(r)

                # Mg[t, w] = (seg[t] == w) * g[t], fp16, t-chunked, on DVE
                # (gpsimd tensor_scalar is a ~4us DSP program -- never use it)
                mbar = m_pool.tile([128, 2, T], HP)
                for c in range(2):
                    nc.vector.tensor_scalar(
                        mbar[:, c, :],
                        iota_f[:],
                        segt[:, c, r : r + 1],
                        gt[:, c, r : r + 1],
                        eq,
                        mult,
                    )

                # stage 1: logits for both t-chunks into one fp32 PSUM tile
                psl = psl_pool.tile([128, 2, S], F32)
                for c in range(2):
                    for k in range(KCH):
                        nc.tensor.matmul(
                            psl[:, c, :],
                            ht[:, k, 128 * c : 128 * (c + 1)],
                            wt[:, k, :],
                            start=(k == 0),
                            stop=(k == KCH - 1 and not with_bias),
                        )
                    if with_bias:
                        nc.tensor.matmul(
                            psl[:, c, :], ones[:], bsb[:], start=False, stop=True
                        )

                # stage 2 of the PREVIOUS row goes on the PE queue here so the
                # PE never waits on the ACT-produced lsb of the same row
                if pending is not None:
                    emit_stage2(pending)

                # PSUM -> SBUF fp16 in one ACT copy (g lives in Mg, not here)
                lsb = l_pool.tile([128, 2, S], HP)
                nc.scalar.copy(lsb[:], psl[:])
                pending = (r, lsb, mbar)
            emit_stage2(pending)

    if split_waits:
        _split_sync_waits(nc)
    return nc


def _host_prep(hidden, W, b, seg):
    """Pure layout/encoding prep (no float arithmetic on the model data
    beyond 1/count of the integer segment ids)."""
    # [core][p, r, k, t] with p the SBUF partition (= h % 128 within chunk k)
    import ml_dtypes

    h8 = np.asarray(hidden[:, 1:, :], dtype=np.float32).astype(ml_dtypes.float8_e3m4)
    h8 = h8.reshape(N_CORES, RPC, T, KCH, 128)
    hiddenT = np.ascontiguousarray(h8.transpose(0, 4, 1, 3, 2))

    seg = np.asarray(seg)
    counts = np.zeros((B, T), dtype=np.int64)
    rows = np.arange(B)[:, None]
    np.add.at(counts, (rows, seg), 1)
    g = (1.0 / np.maximum(counts, 1))[rows, seg].astype(np.float32)  # [B, T]
    segf = seg.astype(np.float32)

    # partition-major packing: [core][p, c, r] = value at (row0+r, 128c+p)
    def pack(x):
        # x: [B, T] -> [N_CORES, 128, 2, RPC]
        x4 = x.reshape(N_CORES, RPC, 2, 128)  # [core, r, c, p]
        return np.ascontiguousarray(x4.transpose(0, 3, 2, 1))

    segt = pack(segf)
    gt = pack(g)
    w16 = np.asarray(W, dtype=np.float32).astype(np.float16).reshape(KCH, 128, S)
    w_in = np.ascontiguousarray(w16.transpose(1, 0, 2))  # [128, KCH, S]
    b_in = np.ascontiguousarray(b, dtype=np.float32).astype(np.float16).reshape(1, S)
    return hiddenT, w_in, b_in, segt, gt


_CACHE = {}


def kernel(hidden, W, b, seg):
    hiddenT, w_in, b_in, segt, gt = _host_prep(hidden, W, b, seg)
    with_bias = bool(np.any(b_in != 0.0))

    key = ("prog", with_bias)
    if key not in _CACHE:
        _CACHE[key] = _build_program(with_bias=with_bias)
    nc = _CACHE[key]

    in_maps = []
    for c in range(N_CORES):
        in_maps.append(
            {
                "hiddent": hiddenT[c],
                "w": w_in,
                "bvec": b_in,
                "segt": segt[c],
                "gt": gt[c],
            }
        )
    res = run_bass_kernel_spmd(nc, in_maps, core_ids=list(range(N_CORES)))
    # device layout is [w_part=128, RPC, w_chunk=2, S]; out[b, s, 128*wc + p]
    # = dev[p, r, wc, s] -> transpose to [RPC, S, wc, p] and flatten t.
    parts = []
    for c in range(N_CORES):
        dev = res.results[c]["out"]  # [128, RPC, 2, S] fp16
        parts.append(
            dev.transpose(1, 3, 2, 0).reshape(RPC, S, T).astype(np.float32)
        )
    return np.ascontiguousarray(np.concatenate(parts, axis=0))

